# revision 5
# baseline (speedup 1.0000x reference)
"""Trainium2 Bass kernel for 16-head causal MHA (B=4, S=2048, E=1024, D=64).

Sharding: 8 cores = 4 batches x 2 head-halves. Each core computes QKV
projections + causal attention for 8 heads of one batch plus the partial
output projection for its head-half's columns of Wo. Host sums the two
partials per batch and adds the effective bias (bo + bv-through-Wo, since
softmax rows sum to 1 the V-bias contribution is a constant vector).

All matmuls run in fp32r (full PE rate, ~1e-4 relative rounding).
"""
import numpy as np

B, S, E = 4, 2048, 1024
H, D = 16, 64
NP = 4     # head-pairs per core (2 heads packed per matmul)
KT = 8     # E / 128 contraction tiles
NQB = 4    # q blocks of 512
NTT = 16   # t tiles of 128

_NC = None


def _build():
    import concourse.bacc as bacc
    import concourse.tile as tile
    from concourse import mybir
    from concourse.masks import make_identity

    f32, f32r = mybir.dt.float32, mybir.dt.float32r
    Act = mybir.ActivationFunctionType

    nc = bacc.Bacc("TRN2")
    X = nc.dram_tensor("x", [S, E], f32, kind="ExternalInput")
    WQ = nc.dram_tensor("wq", [NP, KT, 128, 128], f32, kind="ExternalInput")
    WK = nc.dram_tensor("wk", [NP, KT, 128, 128], f32, kind="ExternalInput")
    WV = nc.dram_tensor("wv", [NP, KT, 128, 128], f32, kind="ExternalInput")
    BQ = nc.dram_tensor("bq", [NP, 128, 1], f32, kind="ExternalInput")
    BK = nc.dram_tensor("bk", [NP, 128, 1], f32, kind="ExternalInput")
    WO = nc.dram_tensor("wo", [NP, 128, E], f32, kind="ExternalInput")
    MS = nc.dram_tensor("masks", [NQB, 128, 512], f32, kind="ExternalInput")
    IND = nc.dram_tensor("ind", [2, 128], f32, kind="ExternalInput")
    SEL = nc.dram_tensor("sel", [128, 4], f32, kind="ExternalInput")
    OUT = nc.dram_tensor("out", [S, E], f32, kind="ExternalOutput")

    with tile.TileContext(nc) as tc:
        with tc.tile_pool(name="persist", bufs=1) as pers:
            ident_f = pers.tile([128, 128], f32)
            make_identity(nc, ident_f)
            ident_r = pers.tile([128, 128], f32r)
            nc.vector.tensor_copy(ident_r, ident_f)
            ones_f = pers.tile([128, 1], f32)
            nc.vector.memset(ones_f, 1.0)
            ones_col = pers.tile([128, 1], f32r)
            nc.vector.tensor_copy(ones_col, ones_f)
            onesrow_f = pers.tile([1, 64], f32)
            nc.vector.memset(onesrow_f, 1.0)
            ones_row = pers.tile([1, 64], f32r)
            nc.vector.tensor_copy(ones_row, onesrow_f)
            # host-supplied constants: 2x128 block indicator + sum selectors
            ind_f = pers.tile([2, 128], f32)
            nc.sync.dma_start(ind_f, IND.ap())
            ind_r = pers.tile([2, 128], f32r)
            nc.vector.tensor_copy(ind_r, ind_f)
            sel_f = pers.tile([128, 4], f32)
            nc.sync.dma_start(sel_f, SEL.ap())
            sel_r = pers.tile([128, 4], f32r)
            nc.vector.tensor_copy(sel_r, sel_f)

            bq_t, bk_t = [], []
            for p in range(NP):
                t1 = pers.tile([128, 1], f32, name=f"bq_t{p}")
                nc.sync.dma_start(t1, BQ.ap()[p])
                bq_t.append(t1)
                t2 = pers.tile([128, 1], f32, name=f"bk_t{p}")
                nc.sync.dma_start(t2, BK.ap()[p])
                bk_t.append(t2)

            QT = [pers.tile([128, S], f32r, name=f"QT{i}") for i in range(NP)]
            KTq = [pers.tile([128, S], f32r, name=f"KTq{i}") for i in range(NP)]
            VN = [pers.tile([128, NTT, 128], f32r, name=f"VN{i}") for i in range(NP)]

            with tc.tile_pool(name="xtp", bufs=1) as xtp:
                xT = [xtp.tile([128, S], f32r, name=f"xT{i}") for i in range(KT)]

                # ---- Phase A: x -> x^T (fp32r) via PE transpose ----
                with tc.tile_pool(name="stA", bufs=3) as sa, \
                     tc.tile_pool(name="psA", bufs=4, space="PSUM") as pA:
                    for st in range(NTT):
                        xrow = sa.tile([128, E], f32)
                        nc.sync.dma_start(xrow, X.ap()[st * 128:(st + 1) * 128, :])
                        for k in range(KT):
                            tp = pA.tile([128, 128], f32)
                            nc.tensor.transpose(tp, xrow[:, k * 128:(k + 1) * 128], ident_f)
                            nc.vector.tensor_copy(xT[k][:, st * 128:(st + 1) * 128], tp)

                # ---- Phase B: QKV projections (transposed, 2-head packed) ----
                with tc.tile_pool(name="stB", bufs=3) as sb_, \
                     tc.tile_pool(name="vt2", bufs=1) as vt2p, \
                     tc.tile_pool(name="psB", bufs=4, space="PSUM") as pB, \
                     tc.tile_pool(name="psBt", bufs=2, space="PSUM") as pBt:
                    for p in range(NP):
                        vt2 = vt2p.tile([128, S], f32r)
                        for W_, bias_, dest in (
                            (WQ, bq_t[p], QT[p]),
                            (WK, bk_t[p], KTq[p]),
                            (WV, None, vt2),
                        ):
                            pss = [pB.tile([128, 512], f32, name="pss") for _ in range(4)]
                            for k in range(KT):
                                wf = sb_.tile([128, 128], f32)
                                nc.sync.dma_start(wf, W_.ap()[p, k])
                                wr = sb_.tile([128, 128], f32r)
                                nc.vector.tensor_copy(wr, wf)
                                for nb in range(4):
                                    nc.tensor.matmul(
                                        pss[nb], wr, xT[k][:, nb * 512:(nb + 1) * 512],
                                        start=(k == 0), stop=(k == KT - 1),
                                    )
                            for nb in range(4):
                                dslc = dest[:, nb * 512:(nb + 1) * 512]
                                if bias_ is not None:
                                    nc.scalar.activation(dslc, pss[nb], Act.Identity, bias=bias_)
                                else:
                                    nc.vector.tensor_copy(dslc, pss[nb])
                        # V back to natural [t, d2] layout
                        for tt in range(NTT):
                            tp = pBt.tile([128, 128], f32r)
                            nc.tensor.transpose(tp, vt2[:, tt * 128:(tt + 1) * 128], ident_r)
                            nc.vector.tensor_copy(VN[p][:, tt, :], tp)

            # xT freed here
            with tc.tile_pool(name="ctxp", bufs=1) as ctxp:
                ctxN = [ctxp.tile([128, S], f32r, name=f"ctxN{i}") for i in range(NP)]

                # ---- Phase C: causal attention ----
                with tc.tile_pool(name="maskp", bufs=1) as maskp, \
                     tc.tile_pool(name="expp", bufs=6) as expp, \
                     tc.tile_pool(name="accp", bufs=4) as accp, \
                     tc.tile_pool(name="rp", bufs=4) as rp, \
                     tc.tile_pool(name="psSC", bufs=4, space="PSUM") as psSC, \
                     tc.tile_pool(name="psCTX", bufs=1, space="PSUM") as psCTX, \
                     tc.tile_pool(name="psSUM", bufs=1, space="PSUM") as psSUM, \
                     tc.tile_pool(name="psRB", bufs=1, space="PSUM") as psRB:
                    maskr = []
                    for j in range(NQB):
                        mf = rp.tile([128, 512], f32)
                        nc.sync.dma_start(mf, MS.ap()[j])
                        mr = maskp.tile([128, 512], f32r, name=f"mask{j}")
                        nc.vector.tensor_copy(mr, mf)
                        maskr.append(mr)

                    for p in range(NP):
                        for qb in range(NQB):
                            T = 4 * (qb + 1)  # causal: t-tiles 0..T-1
                            cps = [psCTX.tile([64, 512], f32, name=f"cps{h}")
                                   for h in range(2)]
                            acc = [accp.tile([128, 512], f32r, name="acc") for _ in range(2)]
                            prev_exp = None
                            for tt in range(T):
                                scs = []
                                for h in range(2):
                                    sc = psSC.tile([128, 512], f32, name="sc")
                                    nc.tensor.matmul(
                                        sc,
                                        KTq[p][h * 64:(h + 1) * 64, tt * 128:(tt + 1) * 128],
                                        QT[p][h * 64:(h + 1) * 64, qb * 512:(qb + 1) * 512],
                                        start=True, stop=True,
                                    )
                                    scs.append(sc)
                                if prev_exp is not None:
                                    for h in range(2):
                                        nc.tensor.matmul(
                                            cps[h],
                                            VN[p][:, tt - 1, h * 64:(h + 1) * 64],
                                            prev_exp[h],
                                            start=(tt - 1 == 0), stop=False,
                                        )
                                cur = []
                                for h in range(2):
                                    ex = expp.tile([128, 512], f32r)
                                    nc.scalar.activation(ex, scs[h], Act.Exp, scale=0.125)
                                    if tt >= 4 * qb:
                                        nc.vector.tensor_mul(ex, ex, maskr[tt - 4 * qb])
                                    if tt == 0:
                                        nc.vector.tensor_copy(acc[h], ex)
                                    else:
                                        nc.vector.tensor_add(acc[h], acc[h], ex)
                                    cur.append(ex)
                                prev_exp = cur
                            for h in range(2):
                                nc.tensor.matmul(
                                    cps[h],
                                    VN[p][:, T - 1, h * 64:(h + 1) * 64],
                                    prev_exp[h],
                                    start=(T - 1 == 0), stop=True,
                                )
                            # softmax denominators: selector colsums -> indicator bcast
                            sm = psSUM.tile([2, 512], f32)
                            nc.tensor.matmul(sm, sel_r[:, 0:2], acc[0], start=True, stop=False)
                            nc.tensor.matmul(sm, sel_r[:, 2:4], acc[1], start=False, stop=True)
                            smr = rp.tile([2, 512], f32r)
                            nc.vector.tensor_copy(smr, sm)
                            rbps = psRB.tile([128, 512], f32)
                            nc.tensor.matmul(rbps, ind_r, smr, start=True, stop=True)
                            rbs = rp.tile([128, 512], f32)
                            nc.vector.reciprocal(rbs, rbps)
                            for h in range(2):
                                nc.vector.tensor_mul(
                                    ctxN[p][h * 64:(h + 1) * 64, qb * 512:(qb + 1) * 512],
                                    cps[h], rbs[h * 64:(h + 1) * 64, :],
                                )

                # ---- Phase D: output projection (partial, this head-half) ----
                with tc.tile_pool(name="stD", bufs=3) as sd, \
                     tc.tile_pool(name="wo2", bufs=1) as wop, \
                     tc.tile_pool(name="psD", bufs=4, space="PSUM") as pD:
                    wo_r = []
                    for p in range(NP):
                        wf = sd.tile([128, E], f32)
                        nc.sync.dma_start(wf, WO.ap()[p])
                        wr = wop.tile([128, E], f32r, name=f"wo2_{p}")
                        nc.vector.tensor_copy(wr, wf)
                        wo_r.append(wr)
                    for qt in range(NTT):
                        ob = sd.tile([128, E], f32)
                        for eh in range(2):
                            ps = pD.tile([128, 512], f32)
                            for p in range(NP):
                                nc.tensor.matmul(
                                    ps,
                                    ctxN[p][:, qt * 128:(qt + 1) * 128],
                                    wo_r[p][:, eh * 512:(eh + 1) * 512],
                                    start=(p == 0), stop=(p == NP - 1),
                                )
                            nc.vector.tensor_copy(ob[:, eh * 512:(eh + 1) * 512], ps)
                        nc.sync.dma_start(OUT.ap()[qt * 128:(qt + 1) * 128, :], ob)

    nc.finalize()
    return nc


def _get_nc():
    global _NC
    if _NC is None:
        _NC = _build()
    return _NC


def _pack_w(Wh):
    # [8, E, D] -> [NP, KT, 128, 128]; out[p,k,i,j] = Wh[2p + j//64, k*128+i, j%64]
    w = Wh.reshape(NP, 2, E, D)
    w = np.transpose(w, (0, 2, 1, 3)).reshape(NP, E, 128)
    w = w.reshape(NP, KT, 128, 128)
    return np.ascontiguousarray(w, dtype=np.float32)


def kernel(x, Wq, bq, Wk, bk, Wv, bv, Wo, bo):
    from concourse.bass_utils import run_bass_kernel_spmd

    x = np.asarray(x, dtype=np.float32)
    Wq = np.asarray(Wq, dtype=np.float32)
    bq = np.asarray(bq, dtype=np.float32)
    Wk = np.asarray(Wk, dtype=np.float32)
    bk = np.asarray(bk, dtype=np.float32)
    Wv = np.asarray(Wv, dtype=np.float32)
    bv = np.asarray(bv, dtype=np.float32)
    Wo = np.asarray(Wo, dtype=np.float32)
    bo = np.asarray(bo, dtype=np.float32)

    nc = _get_nc()

    masks = (np.arange(512)[None, :] >= (128 * np.arange(NQB)[:, None, None]
             + np.arange(128)[None, :, None])).astype(np.float32)
    masks = np.ascontiguousarray(masks)

    ind = np.zeros((2, 128), dtype=np.float32)
    ind[0, 0:64] = 1.0
    ind[1, 64:128] = 1.0
    sel = np.zeros((128, 4), dtype=np.float32)
    sel[:, 0] = 1.0   # head 0 -> row 0
    sel[:, 3] = 1.0   # head 1 -> row 1

    in_maps = []
    for c in range(8):
        b, hh = divmod(c, 2)
        hsel = slice(hh * 8, hh * 8 + 8)
        wo_half = np.ascontiguousarray(
            Wo[:, hh * 512:(hh + 1) * 512].T.reshape(NP, 128, E), dtype=np.float32
        )
        in_maps.append({
            "x": np.ascontiguousarray(x[b]),
            "wq": _pack_w(Wq[hsel]),
            "wk": _pack_w(Wk[hsel]),
            "wv": _pack_w(Wv[hsel]),
            "bq": np.ascontiguousarray(bq[hsel].reshape(NP, 128, 1)),
            "bk": np.ascontiguousarray(bk[hsel].reshape(NP, 128, 1)),
            "wo": wo_half,
            "masks": masks,
            "ind": ind,
            "sel": sel,
        })

    res = run_bass_kernel_spmd(nc, in_maps, core_ids=list(range(8)))
    parts = np.stack([res.results[c]["out"] for c in range(8)])  # [8, S, E]

    # effective bias: bo plus bv routed through Wo (softmax rows sum to 1)
    bo_eff = bo + bv.reshape(-1) @ Wo.T
    out = parts.reshape(B, 2, S, E).sum(axis=1) + bo_eff[None, None, :]
    return out.astype(np.float32)


# revision 9
# speedup vs baseline: 11.3947x; 11.3947x over previous
"""Trainium2 Bass kernel for 16-head causal MHA (B=4, S=2048, E=1024, D=64).

Sharding: 8 cores = 4 batches x 2 head-halves. Each core computes QKV
projections + causal attention for 8 heads of one batch plus the partial
output projection for its head-half's columns of Wo. Host sums the two
partials per batch and adds the effective bias (bo + bv-through-Wo, since
softmax rows sum to 1 the V-bias contribution is a constant vector).

All matmuls run in fp32r (full PE rate, ~1e-4 relative rounding).
V is augmented with a ones column so the ctx matmul's extra output row
accumulates the softmax denominator exactly in PSUM.
"""
import numpy as np

B, S, E = 4, 2048, 1024
H, D = 16, 64
NP = 4     # head-pairs per core (2 heads packed in the transposed projections)
KT = 8     # E / 128 contraction tiles
NQB = 4    # q blocks of 512
NTT = 16   # t tiles of 128

_NC = None


def _build():
    import concourse.bacc as bacc
    import concourse.tile as tile
    from concourse import mybir
    from concourse.masks import make_identity

    f32, f32r = mybir.dt.float32, mybir.dt.float32r
    Act = mybir.ActivationFunctionType

    nc = bacc.Bacc("TRN2")
    X = nc.dram_tensor("x", [S, E], f32, kind="ExternalInput")
    WQ = nc.dram_tensor("wq", [NP, KT, 128, 128], f32, kind="ExternalInput")
    WK = nc.dram_tensor("wk", [NP, KT, 128, 128], f32, kind="ExternalInput")
    WV = nc.dram_tensor("wv", [NP, KT, 128, 128], f32, kind="ExternalInput")
    BQ = nc.dram_tensor("bq", [NP, 128, 1], f32, kind="ExternalInput")
    BK = nc.dram_tensor("bk", [NP, 128, 1], f32, kind="ExternalInput")
    WO = nc.dram_tensor("wo", [NP, 128, E], f32, kind="ExternalInput")
    TRI = nc.dram_tensor("tri", [128, 128], f32, kind="ExternalInput")
    OUT = nc.dram_tensor("out", [S, E], f32, kind="ExternalOutput")

    with tile.TileContext(nc) as tc:
        with tc.tile_pool(name="persist", bufs=1) as pers:
            ident_f = pers.tile([128, 128], f32)
            make_identity(nc, ident_f)
            ident_r = pers.tile([128, 128], f32r)
            nc.vector.tensor_copy(ident_r, ident_f)
            ones_f = pers.tile([128, 1], f32)
            nc.vector.memset(ones_f, 1.0)
            onesrow_f = pers.tile([1, 64], f32)
            nc.vector.memset(onesrow_f, 1.0)
            ones_row = pers.tile([1, 64], f32r)
            nc.vector.tensor_copy(ones_row, onesrow_f)
            tri_f = pers.tile([128, 128], f32)
            nc.sync.dma_start(tri_f, TRI.ap())
            tri_r = pers.tile([128, 128], f32r)
            nc.vector.tensor_copy(tri_r, tri_f)
            zeros_f = pers.tile([128, 384], f32)
            nc.vector.memset(zeros_f, 0.0)
            zeros_r = pers.tile([128, 384], f32r)
            nc.vector.tensor_copy(zeros_r, zeros_f)

            bq_t, bk_t = [], []
            for p in range(NP):
                t1 = pers.tile([128, 1], f32, name=f"bq_t{p}")
                nc.sync.dma_start(t1, BQ.ap()[p])
                bq_t.append(t1)
                t2 = pers.tile([128, 1], f32, name=f"bk_t{p}")
                nc.sync.dma_start(t2, BK.ap()[p])
                bk_t.append(t2)

            with tc.tile_pool(name="xtp", bufs=1) as xtp, \
                 tc.tile_pool(name="ctxp", bufs=1) as ctxp:
                xT = [xtp.tile([128, S], f32r, name=f"xT{i}") for i in range(KT)]
                ctxN = [ctxp.tile([128, S], f32r, name=f"ctxN{i}") for i in range(NP)]

                # ---- Phase A: x -> x^T (fp32r) via PE transpose ----
                # column-major load: xT[k] completes after one 1MB DMA, so
                # the k-ordered QKV matmuls can start almost immediately
                xcols = X.ap().rearrange("(st p) e -> p st e", p=128)
                with tc.tile_pool(name="stA", bufs=2) as sa, \
                     tc.tile_pool(name="psA", bufs=4, space="PSUM") as pA:
                    for k in range(KT):
                        colblk = sa.tile([128, NTT, 128], f32)
                        nc.sync.dma_start(colblk, xcols[:, :, k * 128:(k + 1) * 128])
                        for st in range(NTT):
                            tp = pA.tile([128, 128], f32)
                            nc.tensor.transpose(tp, colblk[:, st, :], ident_f)
                            nc.vector.tensor_copy(xT[k][:, st * 128:(st + 1) * 128], tp)

                # ---- Phases B+C merged: per pair, QKV projection then attention ----
                with tc.tile_pool(name="qtp", bufs=2) as qtp, \
                     tc.tile_pool(name="ktp", bufs=2) as ktp, \
                     tc.tile_pool(name="vnp", bufs=2) as vnp, \
                     tc.tile_pool(name="stB", bufs=3) as sb_, \
                     tc.tile_pool(name="vt2", bufs=1) as vt2p, \
                     tc.tile_pool(name="expp", bufs=6) as expp, \
                     tc.tile_pool(name="rp", bufs=4) as rp, \
                     tc.tile_pool(name="psB", bufs=4, space="PSUM") as pB, \
                     tc.tile_pool(name="psCTX", bufs=1, space="PSUM") as psCTX:
                    for p in range(NP):
                        qt = qtp.tile([128, S], f32r, name="qt")
                        kt = ktp.tile([128, S], f32r, name="kt")
                        vn = vnp.tile([128, 2, NTT, 65], f32r, name="vn")
                        vt2 = vt2p.tile([128, S], f32r)

                        # QKV projections (transposed, 2-head packed)
                        for W_, bias_, dest in (
                            (WQ, bq_t[p], qt),
                            (WK, bk_t[p], kt),
                            (WV, None, vt2),
                        ):
                            wrs = []
                            for k in range(KT):
                                wf = sb_.tile([128, 128], f32, name="wf", bufs=3)
                                nc.sync.dma_start(wf, W_.ap()[p, k])
                                wr = sb_.tile([128, 128], f32r, name="wr", bufs=10)
                                nc.vector.tensor_copy(wr, wf)
                                wrs.append(wr)
                            for half in range(2):
                                pss = [pB.tile([128, 512], f32, name="pss", bufs=2)
                                       for _ in range(2)]
                                for k in range(KT):
                                    for i in range(2):
                                        nb = 2 * half + i
                                        nc.tensor.matmul(
                                            pss[i], wrs[k],
                                            xT[k][:, nb * 512:(nb + 1) * 512],
                                            start=(k == 0), stop=(k == KT - 1),
                                        )
                                for i in range(2):
                                    nb = 2 * half + i
                                    dslc = dest[:, nb * 512:(nb + 1) * 512]
                                    if bias_ is not None:
                                        nc.vector.tensor_scalar_add(dslc, pss[i], bias_)
                                    else:
                                        nc.vector.tensor_copy(dslc, pss[i])
                        # V back to natural [t, d] layout, split per head + ones col
                        for tt in range(NTT):
                            tp2 = pB.tile([128, 128], f32r, name="sc", bufs=4)
                            nc.tensor.transpose(tp2, vt2[:, tt * 128:(tt + 1) * 128], ident_r)
                            for h in range(2):
                                nc.vector.tensor_copy(
                                    vn[:, h, tt, 0:64], tp2[:, h * 64:(h + 1) * 64])
                                nc.vector.tensor_copy(vn[:, h, tt, 64:65], ones_f)

                        # attention for this pair
                        for qb in range(NQB):
                            T = 4 * (qb + 1)  # causal: t-tiles 0..T-1
                            cps = [psCTX.tile([65, 512], f32, name=f"cps{h}")
                                   for h in range(2)]
                            prev_exp = None
                            for tt in range(T):
                                scs = []
                                for h in range(2):
                                    sc = pB.tile([128, 512], f32, name="sc", bufs=4)
                                    nc.tensor.matmul(
                                        sc,
                                        kt[h * 64:(h + 1) * 64, tt * 128:(tt + 1) * 128],
                                        qt[h * 64:(h + 1) * 64, qb * 512:(qb + 1) * 512],
                                        start=True, stop=True,
                                    )
                                    scs.append(sc)
                                if prev_exp is not None:
                                    for h in range(2):
                                        nc.tensor.matmul(
                                            cps[h], vn[:, h, tt - 1, :], prev_exp[h],
                                            start=(tt - 1 == 0), stop=False,
                                        )
                                j = tt - 4 * qb  # >=0 on diagonal tiles
                                cur = []
                                for h in range(2):
                                    ex = expp.tile([128, 512], f32r)
                                    if j >= 1:
                                        nc.gpsimd.tensor_copy(
                                            ex[:, 0:j * 128], zeros_r[:, 0:j * 128])
                                    if j >= 0:
                                        nc.scalar.activation(
                                            ex[:, j * 128:512], scs[h][:, j * 128:512],
                                            Act.Exp, scale=0.125)
                                        nc.vector.tensor_mul(
                                            ex[:, j * 128:(j + 1) * 128],
                                            ex[:, j * 128:(j + 1) * 128], tri_r)
                                    else:
                                        nc.scalar.activation(ex, scs[h], Act.Exp, scale=0.125)
                                    cur.append(ex)
                                prev_exp = cur
                            for h in range(2):
                                nc.tensor.matmul(
                                    cps[h], vn[:, h, T - 1, :], prev_exp[h],
                                    start=(T - 1 == 0), stop=True,
                                )
                            # denominators (row 64 of cps) -> bcast -> normalize
                            for h in range(2):
                                rh = rp.tile([1, 512], f32r, name="rh")
                                nc.vector.tensor_copy(rh, cps[h][64:65, :])
                                rb = pB.tile([64, 512], f32, name="sc", bufs=4)
                                nc.tensor.matmul(rb, ones_row, rh, start=True, stop=True)
                                rbs = rp.tile([64, 512], f32, name="rbs")
                                nc.vector.reciprocal(rbs, rb)
                                nc.vector.tensor_mul(
                                    ctxN[p][h * 64:(h + 1) * 64, qb * 512:(qb + 1) * 512],
                                    cps[h][0:64, :], rbs,
                                )

                # ---- Phase D: output projection (partial, this head-half) ----
                with tc.tile_pool(name="stD", bufs=3) as sd, \
                     tc.tile_pool(name="wo2", bufs=1) as wop, \
                     tc.tile_pool(name="psD", bufs=4, space="PSUM") as pD:
                    wo_r = []
                    for p in range(NP):
                        wf2 = sd.tile([128, E], f32, name="wf2")
                        nc.sync.dma_start(wf2, WO.ap()[p])
                        wr2 = wop.tile([128, E], f32r, name=f"wo2_{p}")
                        nc.vector.tensor_copy(wr2, wf2)
                        wo_r.append(wr2)
                    for qt_i in range(NTT):
                        ob = sd.tile([128, E], f32, name="ob")
                        for eh in range(2):
                            ps = pD.tile([128, 512], f32, name="psd")
                            for p in range(NP):
                                nc.tensor.matmul(
                                    ps,
                                    ctxN[p][:, qt_i * 128:(qt_i + 1) * 128],
                                    wo_r[p][:, eh * 512:(eh + 1) * 512],
                                    start=(p == 0), stop=(p == NP - 1),
                                )
                            nc.vector.tensor_copy(ob[:, eh * 512:(eh + 1) * 512], ps)
                        nc.sync.dma_start(OUT.ap()[qt_i * 128:(qt_i + 1) * 128, :], ob)

    nc.finalize()
    return nc


def _get_nc():
    global _NC
    if _NC is None:
        _NC = _build()
    return _NC


def _pack_w(Wh):
    # [8, E, D] -> [NP, KT, 128, 128]; out[p,k,i,j] = Wh[2p + j//64, k*128+i, j%64]
    w = Wh.reshape(NP, 2, E, D)
    w = np.transpose(w, (0, 2, 1, 3)).reshape(NP, E, 128)
    w = w.reshape(NP, KT, 128, 128)
    return np.ascontiguousarray(w, dtype=np.float32)


def kernel(x, Wq, bq, Wk, bk, Wv, bv, Wo, bo):
    from concourse.bass_utils import run_bass_kernel_spmd

    x = np.asarray(x, dtype=np.float32)
    Wq = np.asarray(Wq, dtype=np.float32)
    bq = np.asarray(bq, dtype=np.float32)
    Wk = np.asarray(Wk, dtype=np.float32)
    bk = np.asarray(bk, dtype=np.float32)
    Wv = np.asarray(Wv, dtype=np.float32)
    bv = np.asarray(bv, dtype=np.float32)
    Wo = np.asarray(Wo, dtype=np.float32)
    bo = np.asarray(bo, dtype=np.float32)

    nc = _get_nc()

    tri = (np.arange(128)[None, :] >= np.arange(128)[:, None]).astype(np.float32)
    tri = np.ascontiguousarray(tri)

    in_maps = []
    for c in range(8):
        b, hh = divmod(c, 2)
        hsel = slice(hh * 8, hh * 8 + 8)
        wo_half = np.ascontiguousarray(
            Wo[:, hh * 512:(hh + 1) * 512].T.reshape(NP, 128, E), dtype=np.float32
        )
        in_maps.append({
            "x": np.ascontiguousarray(x[b]),
            "wq": _pack_w(Wq[hsel]),
            "wk": _pack_w(Wk[hsel]),
            "wv": _pack_w(Wv[hsel]),
            "bq": np.ascontiguousarray(bq[hsel].reshape(NP, 128, 1)),
            "bk": np.ascontiguousarray(bk[hsel].reshape(NP, 128, 1)),
            "wo": wo_half,
            "tri": tri,
        })

    res = run_bass_kernel_spmd(nc, in_maps, core_ids=list(range(8)))
    parts = np.stack([res.results[c]["out"] for c in range(8)])  # [8, S, E]

    # effective bias: bo plus bv routed through Wo (softmax rows sum to 1)
    bo_eff = bo + bv.reshape(-1) @ Wo.T
    out = parts.reshape(B, 2, S, E).sum(axis=1) + bo_eff[None, None, :]
    return out.astype(np.float32)
